# revision 42
# baseline (speedup 1.0000x reference)
"""Trainium2 Bass kernel for nn_DIMPA (3-hop dual-graph COO SpMM).

Strategy (8 NeuronCores, SPMD single program):
  - Destination nodes sharded across cores (12500 rows/core).
  - Host pre-sorts each core's edges by (dest-tile-of-128, src-quartile),
    builds int16 gather indices (quartile-relative, so they fit int16),
    f32 edge values and f32 local-dest ids, laid out per 128-edge chunk.
  - Device, per dest tile: SWDGE dma_gather of source rows from HBM,
    DVE builds a one-hot "segment matrix" (iota == dst_local) and scales
    gathered rows by edge values, PE computes onehot.T @ feats which IS
    the segment-sum (scatter-add) into PSUM, accumulated over chunks.
  - feat accumulators (w[h] * curr_h) live in SBUF for the whole kernel.
  - After hops 1 and 2, an AllGather rebuilds the full N x D "curr" in
    each core's HBM to serve as the next hop's gather source.
"""

import math
from contextlib import ExitStack

import numpy as np

import concourse.bass as bass
import concourse.bacc as bacc
import concourse.tile as tile
from concourse import library_config, mybir
from concourse.bass_utils import run_bass_kernel_spmd

F32 = mybir.dt.float32
BF16 = mybir.dt.bfloat16
I16 = mybir.dt.int16
I32 = mybir.dt.int32


class Cfg:
    def __init__(self, N=100000, E=1200000, D=64, HOP=3, CORES=8, NQ=4,
                 debug=False, mm_bf16=False, cnt_reg=False, host_oh=False):
        assert N % CORES == 0 and N % NQ == 0
        self.N, self.E, self.D, self.HOP, self.CORES, self.NQ = N, E, D, HOP, CORES, NQ
        self.NPC = N // CORES              # nodes per core
        self.TILES = math.ceil(self.NPC / 128)
        self.TAIL = self.NPC - (self.TILES - 1) * 128
        self.QROWS = N // NQ               # rows per source quartile
        assert self.QROWS <= 32767, "gather idx must fit int16"
        self.debug = debug
        self.mm_bf16 = mm_bf16             # bf16 matmul operands (FWL)
        self.cnt_reg = cnt_reg             # runtime valid-count per gather
        self.host_oh = host_oh             # host-baked val-scaled bf16 onehot
        self.mock_cc = False               # timing-sim only: no collectives


def _preprocess_graph(cfg, rows, cols, vals):
    """Per-core edge layout. Edges keyed by (dest-tile, src-quartile,
    dest-half): each 128-edge chunk targets one 64-row half of the dest tile
    so the one-hot segment matrix is only 64 wide and LDWEIGHTS is 64 cols.
    One gather call per (tile, quartile) covers its h0+h1 chunks
    contiguously."""
    NQ, T = cfg.NQ, cfg.TILES
    NCELL = T * NQ * 2                     # (t, q, h) cells
    rows = np.asarray(rows); cols = np.asarray(cols); vals = np.asarray(vals)
    core = rows // cfg.NPC
    per_core = []
    for c in range(cfg.CORES):
        sel = core == c
        r = rows[sel] - c * cfg.NPC
        s = cols[sel]
        v = vals[sel]
        t = r // 128
        dl = r % 128
        h = dl // 64
        q = s // cfg.QROWS
        i16 = (s % cfg.QROWS).astype(np.int16)
        key = (t * NQ + q) * 2 + h
        order = np.argsort(key, kind="stable")
        per_core.append((key[order], i16[order], v[order].astype(np.float32),
                         (dl[order] % 64).astype(np.float32)))
    counts = [np.bincount(k, minlength=NCELL) for k, _, _, _ in per_core]

    # schedule: chunks per (t, q, h), same for all cores
    mx = np.max(np.stack(counts, 0), axis=0).reshape(T, NQ, 2)
    kq = -(-mx // 128)                     # ceil
    for t in range(T):                     # every psum half needs >=1 chunk
        for h in range(2):
            if kq[t, :, h].sum() == 0:
                kq[t, 0, h] = 1

    kt = kq.sum(axis=(1, 2))                          # chunks per tile
    tb = np.concatenate([[0], np.cumsum(kt)])         # tile chunk base
    TC = int(tb[-1])
    # chunk offset of (q, h) within tile: q-major, then h
    qoff = np.zeros((T, NQ, 2), np.int64)
    halves = []                                       # per tile: half of chunk
    for t in range(T):
        off = 0
        hs = []
        for q in range(NQ):
            for h in range(2):
                qoff[t, q, h] = off
                off += kq[t, q, h]
                hs += [h] * int(kq[t, q, h])
        halves.append(hs)
    # idx columns per (t,q) call: 8 cols per chunk
    kq_call = kq.sum(axis=2)                          # [T, NQ]
    ib = np.concatenate([[0], np.cumsum(kq_call.reshape(-1) * 8)])
    IC = int(ib[-1])

    call_of = -np.ones(T * NQ, np.int64)
    ncalls = 0
    for t in range(T):
        for q in range(NQ):
            if kq_call[t, q] > 0:
                call_of[t * NQ + q] = ncalls
                ncalls += 1

    import ml_dtypes
    core_arrays = []
    for (key, i16, v, dl64), cnts in zip(per_core, counts):
        val_dev = np.zeros((128, TC), np.float32)
        dst_dev = np.zeros((128, TC), np.float32)
        oh_dev = (np.zeros((128, TC * 64), ml_dtypes.bfloat16)
                  if cfg.host_oh else None)
        idx_dev = np.zeros((128, IC), np.int16)
        cnts2 = cnts.reshape(T, NQ, 2)
        if len(key):
            gstart = np.concatenate([[0], np.cumsum(cnts)])[:-1]
            j = np.arange(len(key)) - gstart[key]     # pos within cell
            tt = key // (NQ * 2)
            qq = (key // 2) % NQ
            hh = key % 2
            gchunk = tb[tt] + qoff[tt, qq, hh] + j // 128
            lane = j % 128
            val_dev[lane, gchunk] = v
            dst_dev[lane, gchunk] = dl64
            if oh_dev is not None:
                oh_dev[lane, gchunk * 64 + dl64.astype(np.int64)] = v
            # idx position within the (t,q) call: h0 block then h1 block
            jc = j + (hh * kq[tt, qq, 0] * 128)
            col = ib[tt * NQ + qq] + jc // 16
            part = (jc % 16).astype(np.int64)
            for g in range(8):
                idx_dev[part + 16 * g, col] = i16
        cnt_dev = np.zeros((1, max(ncalls, 1)), np.int32)
        if cfg.cnt_reg:
            # mark the h1 tail invalid (-1); h0 pads stay idx 0 (mid-call
            # negatives are not allowed)
            for t in range(T):
                for q in range(NQ):
                    ci = call_of[t * NQ + q]
                    if ci < 0:
                        continue
                    k0 = int(kq[t, q, 0])
                    k1 = int(kq[t, q, 1])
                    n1 = int(cnts2[t, q, 1])
                    base = k0 * 128
                    if k1 > 0 and n1 < k1 * 128:
                        if n1 == 0:
                            n1 = 1        # keep a valid tail descriptor
                        lo = base + n1
                        jj = np.arange(lo, base + k1 * 128)
                        colp = ib[t * NQ + q] + jj // 16
                        pp = jj % 16
                        for g in range(8):
                            idx_dev[pp + 16 * g, colp] = -1
                    cnt_dev[0, ci] = base + n1 if k1 > 0 else base
    # degenerate: calls with k0==0 handled implicitly (base=0, h1 logic)
        arrs = {"idx": idx_dev, "val": val_dev, "dst": dst_dev,
                "cnt": cnt_dev}
        if oh_dev is not None:
            arrs["oh"] = oh_dev
        core_arrays.append(arrs)
    meta = {"kq": kq, "kt": kt, "tb": tb, "TC": TC, "qoff": qoff,
            "kq_call": kq_call, "halves": halves,
            "ib": ib.reshape(-1), "IC": IC, "call_of": call_of,
            "ncalls": max(ncalls, 1)}
    return meta, core_arrays


def build_program(cfg, meta_s, meta_t):
    nc = bacc.Bacc("TRN2", target_bir_lowering=False, debug=cfg.debug,
                   num_devices=cfg.CORES)
    N, D, HOP, TILES, TAIL = cfg.N, cfg.D, cfg.HOP, cfg.TILES, cfg.TAIL
    NPC, NQ, QROWS = cfg.NPC, cfg.NQ, cfg.QROWS
    graphs = ("s", "t")
    metas = {"s": meta_s, "t": meta_t}

    # ---- I/O ----
    # host_oh mode: gather sources are bf16 padded to 2D columns so each
    # gathered row is 256 B; one-hots come val-scaled from the host.
    src_dt = BF16 if cfg.host_oh else F32
    SRCW = 2 * D if cfg.host_oh else D
    xfull = {g: nc.dram_tensor(f"xfull_{g}", [N, SRCW], src_dt,
                               kind="ExternalInput") for g in graphs}
    xown = {g: nc.dram_tensor(f"xown_{g}", [TILES * 128, D], F32,
                              kind="ExternalInput") for g in graphs}
    idx_d = {g: nc.dram_tensor(f"idx_{g}", [128, metas[g]["IC"]], I16,
                               kind="ExternalInput") for g in graphs}
    if cfg.host_oh:
        oh_d = {g: nc.dram_tensor(f"oh_{g}", [128, metas[g]["TC"] * 64],
                                  BF16, kind="ExternalInput") for g in graphs}
    else:
        val_d = {g: nc.dram_tensor(f"val_{g}", [128, metas[g]["TC"]], F32,
                                   kind="ExternalInput") for g in graphs}
        dst_d = {g: nc.dram_tensor(f"dst_{g}", [128, metas[g]["TC"]], F32,
                                   kind="ExternalInput") for g in graphs}
        iota_d = nc.dram_tensor("iotaf", [128, 128], F32,
                                kind="ExternalInput")
    wb_d = {g: nc.dram_tensor(f"wb_{g}", [128, HOP + 1], F32,
                              kind="ExternalInput") for g in graphs}
    cnt_d = {g: nc.dram_tensor(f"cnt_{g}", [1, metas[g]["ncalls"]], I32,
                               kind="ExternalInput") for g in graphs} \
        if cfg.cnt_reg else None
    out_d = nc.dram_tensor("out", [NPC, 2 * D], F32, kind="ExternalOutput")

    # ---- internal DRAM for inter-hop exchange ----
    cur_nxt = {g: {h: nc.dram_tensor(f"curnxt_{g}{h}", [TILES * 128, SRCW],
                                     src_dt)
                   for h in range(1, HOP)} for g in graphs}
    cur_ful = {g: {h: nc.dram_tensor(f"curful_{g}{h}", [N, SRCW], src_dt,
                                     addr_space="Shared")
                   for h in range(1, HOP)} for g in graphs}

    ktmax = max(int(metas[g]["kt"].max()) for g in graphs)

    with tile.TileContext(nc) as tc, ExitStack() as ctx:
        meta_p = ctx.enter_context(tc.tile_pool(name="meta", bufs=1))
        feat_p = ctx.enter_context(tc.tile_pool(name="feat", bufs=1))
        g_p = ctx.enter_context(tc.tile_pool(name="gather", bufs=3))
        oh_p = ctx.enter_context(tc.tile_pool(name="onehot", bufs=3))
        ps_p = ctx.enter_context(tc.tile_pool(name="psum", bufs=4,
                                              space="PSUM"))
        st_p = ctx.enter_context(tc.tile_pool(name="stage", bufs=3))

        nc.gpsimd.load_library(library_config.mlp)

        if not cfg.host_oh:
            iota_f = meta_p.tile([128, 128], F32)
            nc.sync.dma_start(iota_f[:], iota_d[:, :])

        cnt_regs = None
        gt_bufs = None
        if cfg.cnt_reg:
            cnt_regs = [ctx.enter_context(nc.gpsimd.register(f"cntreg{i}"))
                        for i in range(4)]
            # Fixed gather buffers (manual round-robin): skipped (padded)
            # gather rows must read as finite so that 0*val stays 0, so we
            # zero each buffer exactly once up front.
            gt_bufs = [meta_p.tile([128, ktmax, SRCW], src_dt,
                                   name=f"gtbuf{i}")
                       for i in range(3)]
            for b in gt_bufs:
                nc.vector.memset(b[:], 0.0)
        st_bufs = None
        STG = 7                            # tiles per batched staging DMA
        if cfg.host_oh:
            # fixed staging buffers: cols D..2D stay zero (the bf16 pad)
            st_bufs = [meta_p.tile([128, STG, SRCW], src_dt,
                                   name=f"stbuf{i}")
                       for i in range(3)]
            for b in st_bufs:
                nc.vector.memset(b[:], 0.0)

        idx_t, val_t, dst_t, wb_t, feat, cnt_t = {}, {}, {}, {}, {}, {}
        for g in graphs:
            idx_t[g] = meta_p.tile([128, metas[g]["IC"]], I16, tag=f"idx{g}", name=f"idx_t_{g}")
            nc.sync.dma_start(idx_t[g][:], idx_d[g][:, :])
            if not cfg.host_oh:
                val_t[g] = meta_p.tile([128, metas[g]["TC"]], F32, tag=f"val{g}", name=f"val_t_{g}")
                nc.sync.dma_start(val_t[g][:], val_d[g][:, :])
                dst_t[g] = meta_p.tile([128, metas[g]["TC"]], F32, tag=f"dst{g}", name=f"dst_t_{g}")
                nc.sync.dma_start(dst_t[g][:], dst_d[g][:, :])
            wb_t[g] = meta_p.tile([128, HOP + 1], F32, tag=f"wb{g}", name=f"wb_t_{g}")
            nc.sync.dma_start(wb_t[g][:], wb_d[g][:, :])
            if cfg.cnt_reg:
                cnt_t[g] = meta_p.tile([1, metas[g]["ncalls"]], I32,
                                       tag=f"cnt{g}", name=f"cnt_t_{g}")
                nc.sync.dma_start(cnt_t[g][:], cnt_d[g][:, :])
            # feat init: feat = w[0] * x_own
            feat[g] = feat_p.tile([128, TILES, D], F32, tag=f"feat{g}", name=f"feat_{g}")
            nc.sync.dma_start(
                feat[g][:],
                xown[g].ap().rearrange("(t p) d -> p t d", p=128))
            nc.vector.tensor_scalar_mul(
                feat[g][:].rearrange("p t d -> p (t d)"),
                feat[g][:].rearrange("p t d -> p (t d)"),
                wb_t[g][:, 0:1])

        tile_rr = 0
        st_rr = 0
        OG = 4                             # tiles per batched one-hot DMA
        for h in range(1, HOP + 1):
            for g in graphs:
                m = metas[g]
                src = xfull[g] if h == 1 else cur_ful[g][h - 1]
                for t in range(TILES):
                    kt = int(m["kt"][t])
                    halves = m["halves"][t]
                    if cfg.cnt_reg:
                        gt = gt_bufs[tile_rr % 3][:, :kt, :]
                        tile_rr += 1
                    else:
                        gt = g_p.tile([128, kt, SRCW], src_dt, tag="gt")
                    for q in range(NQ):
                        kq = int(m["kq_call"][t, q])
                        if kq == 0:
                            continue
                        qo = int(m["qoff"][t, q, 0])
                        ibase = int(m["ib"][t * NQ + q])
                        if cfg.cnt_reg:
                            ci = int(m["call_of"][t * NQ + q])
                            reg = cnt_regs[ci % 4]
                            nc.gpsimd.reg_load(reg, cnt_t[g][0:1, ci:ci + 1])
                            nreg = reg
                        else:
                            nreg = kq * 128
                        nc.gpsimd.dma_gather(
                            gt[:, qo:qo + kq, :],
                            src[q * QROWS:(q + 1) * QROWS, :],
                            idx_t[g][:, ibase:ibase + kq * 8],
                            kq * 128, nreg, SRCW)
                    tb = int(m["tb"][t])
                    if cfg.host_oh:
                        if t % OG == 0:
                            gend = int(m["tb"][min(t + OG, TILES)])
                            ohg = oh_p.tile([128, gend - tb, 64], BF16,
                                            tag="oh", name="ohg")
                            nc.scalar.dma_start(
                                ohg[:],
                                oh_d[g][:, tb * 64:gend * 64])
                            ohg_base = tb
                        oh = ohg[:, tb - ohg_base:tb - ohg_base + kt, :]
                        rhs = gt
                    else:
                        mmdt = BF16 if cfg.mm_bf16 else F32
                        oh = oh_p.tile([128, kt, 64], mmdt, tag="oh")
                        nc.vector.tensor_tensor(
                            oh[:],
                            iota_f[:, 0:64].unsqueeze(1)
                                .broadcast_to([128, kt, 64]),
                            dst_t[g][:, tb:tb + kt].unsqueeze(2)
                                .broadcast_to([128, kt, 64]),
                            mybir.AluOpType.is_equal)
                        if cfg.mm_bf16:
                            rhs = oh_p.tile([128, kt, D], BF16, tag="gtb",
                                            name="gtb")
                        else:
                            rhs = gt
                        nc.vector.tensor_tensor(
                            rhs[:],
                            gt[:],
                            val_t[g][:, tb:tb + kt].unsqueeze(2)
                                .broadcast_to([128, kt, D]),
                            mybir.AluOpType.mult)
                    ps = ps_p.tile([128, D], F32)
                    first = {0: True, 1: True}
                    last_of = {}
                    for c, hc in enumerate(halves):
                        last_of[hc] = c
                    for c, hc in enumerate(halves):
                        nc.tensor.matmul(
                            ps[hc * 64:(hc + 1) * 64, :],
                            oh[:, c, :], rhs[:, c, 0:D],
                            start=first[hc], stop=(c == last_of[hc]),
                            tile_position=(0, hc * 64),
                            skip_group_check=True)
                        first[hc] = False
                    nc.vector.scalar_tensor_tensor(
                        feat[g][:, t, :], ps[:], wb_t[g][:, h:h + 1],
                        feat[g][:, t, :],
                        mybir.AluOpType.mult, mybir.AluOpType.add)
                    if h < HOP:
                        if cfg.host_oh:
                            u = t % STG
                            if u == 0:
                                stg = st_bufs[st_rr % 3]
                                st_rr += 1
                            nc.scalar.copy(stg[:, u, 0:D], ps[:])
                            if u == STG - 1 or t == TILES - 1:
                                t0 = t - u
                                nc.sync.dma_start(
                                    cur_nxt[g][h][t0 * 128:(t + 1) * 128, :]
                                        .rearrange("(u p) d -> p u d", p=128),
                                    stg[:, 0:u + 1, :])
                        else:
                            rows = TAIL if t == TILES - 1 else 128
                            st = st_p.tile([128, D], F32)
                            nc.scalar.copy(st[:], ps[:])
                            nc.sync.dma_start(
                                cur_nxt[g][h][t * 128:t * 128 + rows, :],
                                st[:rows, :])
                if h < HOP:
                    if cfg.mock_cc:
                        # timing-model stand-in for the AllGather: move the
                        # same number of received bytes through the DMA path
                        for r in range(cfg.CORES):
                            nc.sync.dma_start(
                                cur_ful[g][h][r * NPC:(r + 1) * NPC, :],
                                cur_nxt[g][h][0:NPC, :])
                    else:
                        nc.gpsimd.collective_compute(
                            "AllGather", mybir.AluOpType.bypass,
                            replica_groups=[list(range(cfg.CORES))],
                            ins=[cur_nxt[g][h][0:NPC, :].opt()],
                            outs=[cur_ful[g][h].ap().opt()])

        # ---- write output: out[:, 0:D] = feat_s, out[:, D:2D] = feat_t ----
        for g, co in (("s", 0), ("t", D)):
            full_t = TILES - 1
            if full_t > 0:
                nc.sync.dma_start(
                    out_d[0:full_t * 128, co:co + D].rearrange(
                        "(t p) d -> p t d", p=128),
                    feat[g][:, 0:full_t, :])
            nc.sync.dma_start(
                out_d[full_t * 128:NPC, co:co + D],
                feat[g][0:TAIL, full_t, :])

    return nc


def _make_in_maps(cfg, inputs, meta_s, arrs_s, meta_t, arrs_t):
    import ml_dtypes
    x_s = np.asarray(inputs["x_s"], np.float32)
    x_t = np.asarray(inputs["x_t"], np.float32)
    w_s = np.asarray(inputs["w_s"], np.float32)
    w_t = np.asarray(inputs["w_t"], np.float32)
    wb_s = np.tile(w_s.reshape(1, -1), (128, 1)).astype(np.float32)
    wb_t = np.tile(w_t.reshape(1, -1), (128, 1)).astype(np.float32)
    if cfg.host_oh:
        xf_s = np.zeros((cfg.N, 2 * cfg.D), ml_dtypes.bfloat16)
        xf_s[:, :cfg.D] = x_s
        xf_t = np.zeros((cfg.N, 2 * cfg.D), ml_dtypes.bfloat16)
        xf_t[:, :cfg.D] = x_t
    else:
        xf_s, xf_t = x_s, x_t
        iotaf = np.tile(np.arange(128, dtype=np.float32), (128, 1))
    in_maps = []
    for c in range(cfg.CORES):
        xo_s = np.zeros((cfg.TILES * 128, cfg.D), np.float32)
        xo_s[:cfg.NPC] = x_s[c * cfg.NPC:(c + 1) * cfg.NPC]
        xo_t = np.zeros((cfg.TILES * 128, cfg.D), np.float32)
        xo_t[:cfg.NPC] = x_t[c * cfg.NPC:(c + 1) * cfg.NPC]
        im = {
            "xfull_s": xf_s, "xfull_t": xf_t,
            "xown_s": xo_s, "xown_t": xo_t,
            "idx_s": arrs_s[c]["idx"], "idx_t": arrs_t[c]["idx"],
            "wb_s": wb_s, "wb_t": wb_t,
        }
        if cfg.host_oh:
            im["oh_s"] = arrs_s[c]["oh"]
            im["oh_t"] = arrs_t[c]["oh"]
        else:
            im["val_s"] = arrs_s[c]["val"]
            im["dst_s"] = arrs_s[c]["dst"]
            im["val_t"] = arrs_t[c]["val"]
            im["dst_t"] = arrs_t[c]["dst"]
            im["iotaf"] = iotaf
        if cfg.cnt_reg:
            im["cnt_s"] = arrs_s[c]["cnt"]
            im["cnt_t"] = arrs_t[c]["cnt"]
        in_maps.append(im)
    return in_maps


def prepare(cfg, inputs):
    meta_s, arrs_s = _preprocess_graph(
        cfg, inputs["A_rows"], inputs["A_cols"], inputs["A_vals"])
    meta_t, arrs_t = _preprocess_graph(
        cfg, inputs["At_rows"], inputs["At_cols"], inputs["At_vals"])
    nc = build_program(cfg, meta_s, meta_t)
    nc.compile()
    in_maps = _make_in_maps(cfg, inputs, meta_s, arrs_s, meta_t, arrs_t)
    return nc, in_maps


def kernel(**inputs) -> np.ndarray:
    cfg = Cfg()
    nc, in_maps = prepare(cfg, inputs)
    res = run_bass_kernel_spmd(nc, in_maps, list(range(cfg.CORES)))
    return np.concatenate([res.results[c]["out"] for c in range(cfg.CORES)],
                          axis=0)
